# revision 6
# baseline (speedup 1.0000x reference)
"""Cost-volume construction kernel for Trainium2 (8 NeuronCores).

Reference computation (N=1, C=32, H=128, W=240, max_disparity=192, D4=48):
  out[0, c,     i, h, w] = left[0, c, h, w]      if w >= i else 0   (c in [0,32))
  out[0, 32+c,  i, h, w] = right[0, c, h, w-i]   if w >= i else 0

Pure data movement (377 MB output from 8 MB of inputs) -> DMA/HBM-write bound.

Sharding: H is split 8 ways (16 rows per core) so every core runs the exact
same program on its H-slice -- no core-dependent constants needed for SPMD.

Per-core kernel:
  - SBUF partition p = c*4 + (h>>2), free dims (h&3, w). This gives 128
    partitions (full DMA port utilization) while keeping the DRAM dest
    iteration order (c-major, then h, then w) expressible in <=3 AP dims.
  - Right half: one (128, 4, 287) tile, zero-padded in columns [0,47);
    disparity i's output rows are the window [47-i : 287-i) -- a single
    full-row DMA per disparity.
  - Left half: per disparity, a data DMA from the window [i:240) of the
    left tile plus a zeros DMA from a static zero tile for columns [0,i).
    All SBUF tiles are write-once/read-only, so there are no WAR deps.
"""

import numpy as np

C = 32
H = 128
W = 240
D4 = 48
N_CORES = 8
HC = H // N_CORES  # 16 rows per core
PAD = D4 - 1  # 47 zero columns of right-pad

_CACHE = {}


def _build_bass():
    import concourse.bass as bass
    import concourse.mybir as mybir

    f32 = mybir.dt.float32
    nc = bass.Bass(trn_type="TRN2")
    L = nc.dram_tensor("left", (C, HC, W), f32, kind="ExternalInput")
    R = nc.dram_tensor("right", (C, HC, W), f32, kind="ExternalInput")
    Z = nc.dram_tensor("zeros", (C, HC, PAD), f32, kind="ExternalInput")
    O = nc.dram_tensor("out", (2 * C, D4, HC, W), f32, kind="ExternalOutput")

    with (
        nc.sbuf_tensor([128, HC // 4, W], f32) as lt,
        nc.sbuf_tensor([128, HC // 4, W + PAD], f32) as rP,
        nc.Block() as block,
        nc.semaphore("ld_sem") as ld_sem,
        nc.semaphore("st_sem") as st_sem,
    ):
        Lr = L[:].rearrange("c (hq r) w -> (c hq) r w", hq=4)
        Rr = R[:].rearrange("c (hq r) w -> (c hq) r w", hq=4)
        Zr = Z[:].rearrange("c (hq r) w -> (c hq) r w", hq=4)

        @block.sync
        def _(sync):
            # loads: left tile, right-pad zeros, right data
            sync.dma_start(out=lt[:], in_=Lr).then_inc(ld_sem, 16)
            sync.dma_start(out=rP[:, :, 0:PAD], in_=Zr).then_inc(ld_sem, 16)
            sync.dma_start(out=rP[:, :, PAD:], in_=Rr).then_inc(ld_sem, 16)
            sync.wait_ge(ld_sem, 48)

            n_stores = 0
            for i in range(D4):
                # left half: data window + zero prefix (sourced from rP's pad)
                sync.dma_start(out=O[0:C, i, :, i:], in_=lt[:, :, i:]).then_inc(st_sem, 16)
                n_stores += 1
                if i > 0:
                    # i == 1 writes single-element rows; opt() drops the
                    # [1,1] w-dim and the contiguity check complains.
                    with nc.allow_non_contiguous_dma(reason="1-col zero fill"):
                        sync.dma_start(out=O[0:C, i, :, 0:i], in_=rP[:, :, 0:i]).then_inc(st_sem, 16)
                    n_stores += 1
                # right half: pure window slice of the padded tile
                sync.dma_start(
                    out=O[C:, i, :, :], in_=rP[:, :, PAD - i : PAD - i + W]
                ).then_inc(st_sem, 16)
                n_stores += 1
            sync.wait_ge(st_sem, 16 * n_stores)

    return nc


def _get_nc():
    if "nc" not in _CACHE:
        _CACHE["nc"] = _build_bass()
    return _CACHE["nc"]


def _run(in_maps, trace=False):
    from concourse import bass_utils

    return bass_utils.run_bass_kernel_spmd(
        _get_nc(), in_maps, core_ids=list(range(N_CORES)), trace=trace
    )


def kernel(left_feature, right_feature, max_disparity=192, _trace=False):
    assert int(max_disparity) == D4 * 4
    lf = np.ascontiguousarray(np.asarray(left_feature, dtype=np.float32)).reshape(C, H, W)
    rf = np.ascontiguousarray(np.asarray(right_feature, dtype=np.float32)).reshape(C, H, W)

    zeros = np.zeros((C, HC, PAD), dtype=np.float32)
    in_maps = [
        {
            "left": np.ascontiguousarray(lf[:, HC * k : HC * (k + 1), :]),
            "right": np.ascontiguousarray(rf[:, HC * k : HC * (k + 1), :]),
            "zeros": zeros,
        }
        for k in range(N_CORES)
    ]
    res = _run(in_maps, trace=_trace)
    _CACHE["last_result"] = res
    outs = [r["out"] for r in res.results]  # each (64, 48, 16, 240)
    full = np.concatenate(outs, axis=2)  # (64, 48, 128, 240)
    return full.reshape(1, 2 * C, D4, H, W)


# revision 7
# speedup vs baseline: 250.3159x; 250.3159x over previous
"""Cost-volume construction kernel for Trainium2 (8 NeuronCores).

Reference computation (N=1, C=32, H=128, W=240, max_disparity=192, D4=48):
  out[0, c,     i, h, w] = left[0, c, h, w]      if w >= i else 0   (c in [0,32))
  out[0, 32+c,  i, h, w] = right[0, c, h, w-i]   if w >= i else 0

Pure data movement (377 MB output from 8 MB of inputs) -> DMA/HBM-write bound.

Sharding: H is split 8 ways (16 rows per core) so every core runs the exact
same program on its H-slice -- no core-dependent constants needed for SPMD.

Per-core kernel:
  - SBUF partition p = c*4 + (h>>2), free dims (h&3, w). This gives 128
    partitions (full DMA port utilization) while keeping the DRAM dest
    iteration order (c-major, then h, then w) expressible in <=3 AP dims.
  - Right half: one (128, 4, 287) tile, zero-padded in columns [0,47);
    disparity i's output rows are the window [47-i : 287-i) -- a single
    full-row DMA per disparity.
  - Left half: per disparity, a data DMA from the window [i:240) of the
    left tile plus a zeros DMA from a static zero tile for columns [0,i).
    All SBUF tiles are write-once/read-only, so there are no WAR deps.
"""

import numpy as np

C = 32
H = 128
W = 240
D4 = 48
N_CORES = 8
HC = H // N_CORES  # 16 rows per core
PAD = D4 - 1  # 47 zero columns of right-pad

_CACHE = {}


def _build_bass():
    import concourse.bass as bass
    import concourse.mybir as mybir

    f32 = mybir.dt.float32
    nc = bass.Bass(trn_type="TRN2")
    L = nc.dram_tensor("left", (C, HC, W), f32, kind="ExternalInput")
    R = nc.dram_tensor("right", (C, HC, W), f32, kind="ExternalInput")
    Z = nc.dram_tensor("zeros", (C, HC, PAD), f32, kind="ExternalInput")
    O = nc.dram_tensor("out", (2 * C, D4, HC, W), f32, kind="ExternalOutput")

    with (
        nc.sbuf_tensor([128, HC // 4, W], f32) as lt,
        nc.sbuf_tensor([128, HC // 4, W + PAD], f32) as rP,
        nc.Block() as block,
        nc.semaphore("ld_sem") as ld_sem,
        nc.semaphore("st_sem") as st_sem,
    ):
        Lr = L[:].rearrange("c (hq r) w -> (c hq) r w", hq=4)
        Rr = R[:].rearrange("c (hq r) w -> (c hq) r w", hq=4)
        Zr = Z[:].rearrange("c (hq r) w -> (c hq) r w", hq=4)

        @block.sync
        def _(sync):
            # loads: left tile, right-pad zeros, right data
            sync.dma_start(out=lt[:], in_=Lr).then_inc(ld_sem, 16)
            sync.dma_start(out=rP[:, :, 0:PAD], in_=Zr).then_inc(ld_sem, 16)
            sync.dma_start(out=rP[:, :, PAD:], in_=Rr).then_inc(ld_sem, 16)
            sync.wait_ge(ld_sem, 48)

            n_stores = 0
            for i in range(D4):
                # left half: data window + zero prefix (sourced from rP's pad)
                sync.dma_start(out=O[0:C, i, :, i:], in_=lt[:, :, i:]).then_inc(st_sem, 16)
                n_stores += 1
                if i > 0:
                    # i == 1 writes single-element rows; opt() drops the
                    # [1,1] w-dim and the contiguity check complains.
                    with nc.allow_non_contiguous_dma(reason="1-col zero fill"):
                        sync.dma_start(out=O[0:C, i, :, 0:i], in_=rP[:, :, 0:i]).then_inc(st_sem, 16)
                    n_stores += 1
                # right half: pure window slice of the padded tile
                sync.dma_start(
                    out=O[C:, i, :, :], in_=rP[:, :, PAD - i : PAD - i + W]
                ).then_inc(st_sem, 16)
                n_stores += 1
            sync.wait_ge(st_sem, 16 * n_stores)

    return nc


def _get_nc():
    if "nc" not in _CACHE:
        _CACHE["nc"] = _build_bass()
    return _CACHE["nc"]


def _get_exec():
    """Build and cache the jitted SPMD executable (with output donation) and
    a device-side zero-buffer maker, so repeat kernel() calls only pay
    input upload + execution + output download."""
    if "exec" in _CACHE:
        return _CACHE["exec"]

    import jax
    import jax.numpy as jnp
    from jax.sharding import Mesh, NamedSharding, PartitionSpec
    from jax.experimental.shard_map import shard_map
    import concourse.mybir as mybir
    from concourse import bass2jax

    nc = _get_nc()
    bass2jax.install_neuronx_cc_hook()
    partition_name = nc.partition_id_tensor.name if nc.partition_id_tensor else None

    in_names, out_names, out_avals = [], [], []
    for alloc in nc.m.functions[0].allocations:
        if not isinstance(alloc, mybir.MemoryLocationSet):
            continue
        name = alloc.memorylocations[0].name
        if alloc.kind == "ExternalInput":
            if name != partition_name:
                in_names.append(name)
        elif alloc.kind == "ExternalOutput":
            out_names.append(name)
            out_avals.append(
                jax.core.ShapedArray(tuple(alloc.tensor_shape), mybir.dt.np(alloc.dtype))
            )
    n_params = len(in_names)
    all_names = list(in_names) + out_names
    if partition_name is not None:
        all_names.append(partition_name)

    def _body(*args):
        operands = list(args)
        if partition_name is not None:
            operands.append(bass2jax.partition_id_tensor())
        outs = bass2jax._bass_exec_p.bind(
            *operands,
            out_avals=tuple(out_avals),
            in_names=tuple(all_names),
            out_names=tuple(out_names),
            lowering_input_output_aliases=(),
            sim_require_finite=True,
            sim_require_nnan=True,
            nc=nc,
        )
        return tuple(outs)

    devices = jax.devices()[:N_CORES]
    mesh = Mesh(np.asarray(devices), ("core",))
    spec = PartitionSpec("core")
    n_outs = len(out_names)
    donate = tuple(range(n_params, n_params + n_outs))
    fn = jax.jit(
        shard_map(
            _body,
            mesh=mesh,
            in_specs=(spec,) * (n_params + n_outs),
            out_specs=(spec,) * n_outs,
            check_rep=False,
        ),
        donate_argnums=donate,
        keep_unused=True,
    )

    sharding = NamedSharding(mesh, spec)
    zero_makers = [
        jax.jit(
            lambda aval=aval: jnp.zeros((N_CORES * aval.shape[0], *aval.shape[1:]), aval.dtype),
            out_shardings=sharding,
        )
        for aval in out_avals
    ]
    _CACHE["exec"] = (fn, in_names, zero_makers, sharding)
    return _CACHE["exec"]


def kernel(left_feature, right_feature, max_disparity=192):
    import jax

    assert int(max_disparity) == D4 * 4
    lf = np.ascontiguousarray(np.asarray(left_feature, dtype=np.float32)).reshape(C, H, W)
    rf = np.ascontiguousarray(np.asarray(right_feature, dtype=np.float32)).reshape(C, H, W)

    fn, in_names, zero_makers, sharding = _get_exec()
    # global (concat-over-cores) input arrays; core k's shard is its H-slice
    host_in = {
        "left": lf.transpose(1, 0, 2).reshape(N_CORES, HC, C, W).transpose(0, 2, 1, 3).reshape(N_CORES * C, HC, W),
        "right": rf.transpose(1, 0, 2).reshape(N_CORES, HC, C, W).transpose(0, 2, 1, 3).reshape(N_CORES * C, HC, W),
        "zeros": np.zeros((N_CORES * C, HC, PAD), dtype=np.float32),
    }
    args = [jax.device_put(np.ascontiguousarray(host_in[nm]), sharding) for nm in in_names]
    args += [zm() for zm in zero_makers]
    (out_g,) = fn(*args)
    out = np.asarray(out_g)  # (8*64, 48, 16, 240)
    # core k owns H rows [16k, 16k+16): reassemble to (64, 48, 128, 240)
    full = out.reshape(N_CORES, 2 * C, D4, HC, W).transpose(1, 2, 0, 3, 4).reshape(2 * C, D4, H, W)
    return np.ascontiguousarray(full).reshape(1, 2 * C, D4, H, W)


# revision 9
# speedup vs baseline: 266.4770x; 1.0646x over previous
"""Cost-volume construction kernel for Trainium2 (8 NeuronCores).

Reference computation (N=1, C=32, H=128, W=240, max_disparity=192, D4=48):
  out[0, c,     i, h, w] = left[0, c, h, w]      if w >= i else 0   (c in [0,32))
  out[0, 32+c,  i, h, w] = right[0, c, h, w-i]   if w >= i else 0

Pure data movement (377 MB output from 8 MB of inputs) -> DMA/HBM-write bound.

Sharding: H is split 8 ways (16 rows per core) so every core runs the exact
same program on its H-slice -- no core-dependent constants needed for SPMD.

Per-core kernel:
  - SBUF partition p = c*4 + (h>>2), free dims (h&3, w). This gives 128
    partitions (full DMA port utilization) while keeping the DRAM dest
    iteration order (c-major, then h, then w) expressible in <=3 AP dims.
  - Right half: one (128, 4, 287) tile, zero-padded in columns [0,47);
    disparity i's output rows are the window [47-i : 287-i) -- a single
    full-row DMA per disparity.
  - Left half: per disparity, a data DMA from the window [i:240) of the
    left tile plus a zeros DMA from a static zero tile for columns [0,i).
    All SBUF tiles are write-once/read-only, so there are no WAR deps.
"""

import numpy as np

C = 32
H = 128
W = 240
D4 = 48
N_CORES = 8
HC = H // N_CORES  # 16 rows per core
PAD = D4 - 1  # 47 zero columns of right-pad

_CACHE = {}


def _build_bass():
    import concourse.bass as bass
    import concourse.mybir as mybir

    f32 = mybir.dt.float32
    nc = bass.Bass(trn_type="TRN2")
    L = nc.dram_tensor("left", (C, HC, W), f32, kind="ExternalInput")
    R = nc.dram_tensor("right", (C, HC, W), f32, kind="ExternalInput")
    O = nc.dram_tensor("out", (2 * C, D4, HC, W), f32, kind="ExternalOutput")

    with (
        nc.sbuf_tensor([128, HC // 4, W], f32) as lA,
        nc.sbuf_tensor([128, HC // 4, W], f32) as lB,
        nc.sbuf_tensor([128, HC // 4, W + PAD], f32) as rP,
        nc.Block() as block,
        nc.semaphore("ldA") as ldA,
        nc.semaphore("ldB") as ldB,
        nc.semaphore("ldR") as ldR,
        nc.semaphore("dve") as dve,
        nc.semaphore("stA") as stA,
        nc.semaphore("stB") as stB,
        nc.semaphore("stR") as stR,
    ):
        Lr = L[:].rearrange("c (hq r) w -> (c hq) r w", hq=4)
        Rr = R[:].rearrange("c (hq r) w -> (c hq) r w", hq=4)

        # Left half as full 240-col row stores from two alternating buffers
        # (A: even disparities, B: odd). The zero prefix of each buffer grows
        # by DVE memsets of cols [i-2, i) between that buffer's stores.
        # DVE sem ticks: 1 = rP pad memset; i+1 = prefix memset for disp i.

        @block.vector
        def _(vector):
            vector.memset(rP[:, :, 0:PAD], 0.0).then_inc(dve, 1)  # tick 1
            for i in range(1, D4):
                t, sem = (lA, stA) if i % 2 == 0 else (lB, stB)
                lo = max(i - 2, 0)
                if i < 2:
                    vector.wait_ge(ldB, 16)  # first touch of B: only its load
                else:
                    # WAR: all prior stores of this buffer must have drained.
                    n_prior = (i - 2) // 2 + 1
                    vector.wait_ge(sem, 16 * n_prior)
                vector.memset(t[:, :, lo:i], 0.0).then_inc(dve, 1)  # tick i+1

        @block.sync
        def _(sync):
            sync.dma_start(out=lA[:], in_=Lr).then_inc(ldA, 16)
            sync.dma_start(out=lB[:], in_=Lr).then_inc(ldB, 16)
            sync.dma_start(out=rP[:, :, PAD:], in_=Rr).then_inc(ldR, 16)
            sync.wait_ge(ldA, 16)
            for i in range(D4):
                t, sem = (lA, stA) if i % 2 == 0 else (lB, stB)
                if i == 0:
                    sync.dma_start(out=O[0:C, i, :, :], in_=t[:]).then_inc(sem, 16)
                    sync.wait_ge(ldR, 16)
                    sync.wait_ge(dve, 1)  # rP pad memset done
                else:
                    sync.wait_ge(dve, i + 1)  # prefix memset for disp i done
                    sync.dma_start(out=O[0:C, i, :, :], in_=t[:]).then_inc(sem, 16)
                # right half: pure window slice of the padded tile
                sync.dma_start(
                    out=O[C:, i, :, :], in_=rP[:, :, PAD - i : PAD - i + W]
                ).then_inc(stR, 16)
            sync.wait_ge(stA, 16 * (D4 // 2))
            sync.wait_ge(stB, 16 * (D4 // 2))
            sync.wait_ge(stR, 16 * D4)

    return nc


def _get_nc():
    if "nc" not in _CACHE:
        _CACHE["nc"] = _build_bass()
    return _CACHE["nc"]


def _get_exec():
    """Build and cache the jitted SPMD executable (with output donation) and
    a device-side zero-buffer maker, so repeat kernel() calls only pay
    input upload + execution + output download."""
    if "exec" in _CACHE:
        return _CACHE["exec"]

    import jax
    import jax.numpy as jnp
    from jax.sharding import Mesh, NamedSharding, PartitionSpec
    from jax.experimental.shard_map import shard_map
    import concourse.mybir as mybir
    from concourse import bass2jax

    nc = _get_nc()
    bass2jax.install_neuronx_cc_hook()
    partition_name = nc.partition_id_tensor.name if nc.partition_id_tensor else None

    in_names, out_names, out_avals = [], [], []
    for alloc in nc.m.functions[0].allocations:
        if not isinstance(alloc, mybir.MemoryLocationSet):
            continue
        name = alloc.memorylocations[0].name
        if alloc.kind == "ExternalInput":
            if name != partition_name:
                in_names.append(name)
        elif alloc.kind == "ExternalOutput":
            out_names.append(name)
            out_avals.append(
                jax.core.ShapedArray(tuple(alloc.tensor_shape), mybir.dt.np(alloc.dtype))
            )
    n_params = len(in_names)
    all_names = list(in_names) + out_names
    if partition_name is not None:
        all_names.append(partition_name)

    def _body(*args):
        operands = list(args)
        if partition_name is not None:
            operands.append(bass2jax.partition_id_tensor())
        outs = bass2jax._bass_exec_p.bind(
            *operands,
            out_avals=tuple(out_avals),
            in_names=tuple(all_names),
            out_names=tuple(out_names),
            lowering_input_output_aliases=(),
            sim_require_finite=True,
            sim_require_nnan=True,
            nc=nc,
        )
        return tuple(outs)

    devices = jax.devices()[:N_CORES]
    mesh = Mesh(np.asarray(devices), ("core",))
    spec = PartitionSpec("core")
    n_outs = len(out_names)
    donate = tuple(range(n_params, n_params + n_outs))
    fn = jax.jit(
        shard_map(
            _body,
            mesh=mesh,
            in_specs=(spec,) * (n_params + n_outs),
            out_specs=(spec,) * n_outs,
            check_rep=False,
        ),
        donate_argnums=donate,
        keep_unused=True,
    )

    sharding = NamedSharding(mesh, spec)
    zero_makers = [
        jax.jit(
            lambda aval=aval: jnp.zeros((N_CORES * aval.shape[0], *aval.shape[1:]), aval.dtype),
            out_shardings=sharding,
        )
        for aval in out_avals
    ]
    _CACHE["exec"] = (fn, in_names, zero_makers, sharding)
    return _CACHE["exec"]


def kernel(left_feature, right_feature, max_disparity=192):
    import jax

    assert int(max_disparity) == D4 * 4
    lf = np.ascontiguousarray(np.asarray(left_feature, dtype=np.float32)).reshape(C, H, W)
    rf = np.ascontiguousarray(np.asarray(right_feature, dtype=np.float32)).reshape(C, H, W)

    fn, in_names, zero_makers, sharding = _get_exec()
    # global (concat-over-cores) input arrays; core k's shard is its H-slice
    host_in = {
        "left": lf.transpose(1, 0, 2).reshape(N_CORES, HC, C, W).transpose(0, 2, 1, 3).reshape(N_CORES * C, HC, W),
        "right": rf.transpose(1, 0, 2).reshape(N_CORES, HC, C, W).transpose(0, 2, 1, 3).reshape(N_CORES * C, HC, W),
    }
    args = [jax.device_put(np.ascontiguousarray(host_in[nm]), sharding) for nm in in_names]
    args += [zm() for zm in zero_makers]
    (out_g,) = fn(*args)
    out = np.asarray(out_g)  # (8*64, 48, 16, 240)
    # core k owns H rows [16k, 16k+16): reassemble to (64, 48, 128, 240)
    full = out.reshape(N_CORES, 2 * C, D4, HC, W).transpose(1, 2, 0, 3, 4).reshape(2 * C, D4, H, W)
    return np.ascontiguousarray(full).reshape(1, 2 * C, D4, H, W)


# revision 11
# speedup vs baseline: 269.1087x; 1.0099x over previous
"""Cost-volume construction kernel for Trainium2 (8 NeuronCores).

Reference computation (N=1, C=32, H=128, W=240, max_disparity=192, D4=48):
  out[0, c,     i, h, w] = left[0, c, h, w]      if w >= i else 0   (c in [0,32))
  out[0, 32+c,  i, h, w] = right[0, c, h, w-i]   if w >= i else 0

Pure data movement (377 MB output from 8 MB of inputs) -> DMA/HBM-write bound.

Sharding: H is split 8 ways (16 rows per core) so every core runs the exact
same program on its H-slice -- no core-dependent constants needed for SPMD.

Per-core kernel:
  - SBUF partition p = c*4 + (h>>2), free dims (h&3, w). This gives 128
    partitions (full DMA port utilization) while keeping the DRAM dest
    iteration order (c-major, then h, then w) expressible in <=3 AP dims.
  - Right half: one (128, 4, 287) tile, zero-padded in columns [0,47);
    disparity i's output rows are the window [47-i : 287-i) -- a single
    full-row DMA per disparity.
  - Left half: per disparity, a data DMA from the window [i:240) of the
    left tile plus a zeros DMA from a static zero tile for columns [0,i).
    All SBUF tiles are write-once/read-only, so there are no WAR deps.
"""

import numpy as np

C = 32
H = 128
W = 240
D4 = 48
N_CORES = 8
HC = H // N_CORES  # 16 rows per core
PAD = D4 - 1  # 47 zero columns of right-pad

_CACHE = {}


def _build_bass():
    import concourse.bass as bass
    import concourse.mybir as mybir

    f32 = mybir.dt.float32
    nc = bass.Bass(trn_type="TRN2")
    L = nc.dram_tensor("left", (C, HC, W), f32, kind="ExternalInput")
    R = nc.dram_tensor("right", (C, HC, W), f32, kind="ExternalInput")
    O = nc.dram_tensor("out", (2 * C, D4, HC, W), f32, kind="ExternalOutput")

    with (
        nc.sbuf_tensor([128, HC // 4, W], f32) as lA,
        nc.sbuf_tensor([128, HC // 4, W], f32) as lB,
        nc.sbuf_tensor([128, HC // 4, W + PAD], f32) as rP,
        nc.Block() as block,
        nc.semaphore("ldA") as ldA,
        nc.semaphore("ldR") as ldR,
        nc.semaphore("dve") as dve,
        nc.semaphore("stA") as stA,
        nc.semaphore("stB") as stB,
        nc.semaphore("stR") as stR,
    ):
        Lr = L[:].rearrange("c (hq r) w -> (c hq) r w", hq=4)
        Rr = R[:].rearrange("c (hq r) w -> (c hq) r w", hq=4)

        # Left half as full 240-col row stores from two alternating buffers
        # (A: even disparities, B: odd). The zero prefix of each buffer grows
        # by DVE memsets of cols [i-2, i) between that buffer's stores.
        # B is derived from A with an on-chip DVE copy (saves one HBM read).
        # DVE sem ticks: 1 = rP pad memset; 2 = copy A->B; i+2 = prefix(i).

        @block.vector
        def _(vector):
            vector.memset(rP[:, :, 0:PAD], 0.0).then_inc(dve, 1)  # tick 1
            vector.wait_ge(ldA, 16)
            vector.tensor_copy(lB[:], lA[:]).then_inc(dve, 1)  # tick 2
            for i in range(1, D4):
                t, sem = (lA, stA) if i % 2 == 0 else (lB, stB)
                lo = max(i - 2, 0)
                if i >= 2:
                    # WAR: all prior stores of this buffer must have drained.
                    n_prior = (i - 2) // 2 + 1
                    vector.wait_ge(sem, 16 * n_prior)
                vector.memset(t[:, :, lo:i], 0.0).then_inc(dve, 1)  # tick i+2

        @block.sync
        def _(sync):
            sync.dma_start(out=lA[:], in_=Lr).then_inc(ldA, 16)
            sync.dma_start(out=rP[:, :, PAD:], in_=Rr).then_inc(ldR, 16)
            sync.wait_ge(ldA, 16)
            for i in range(D4):
                t, sem = (lA, stA) if i % 2 == 0 else (lB, stB)
                if i == 0:
                    sync.dma_start(out=O[0:C, i, :, :], in_=t[:]).then_inc(sem, 16)
                    sync.wait_ge(ldR, 16)
                    sync.wait_ge(dve, 1)  # rP pad memset done
                else:
                    sync.wait_ge(dve, i + 2)  # copy + prefix memset for disp i
                    sync.dma_start(out=O[0:C, i, :, :], in_=t[:]).then_inc(sem, 16)
                # right half: pure window slice of the padded tile
                sync.dma_start(
                    out=O[C:, i, :, :], in_=rP[:, :, PAD - i : PAD - i + W]
                ).then_inc(stR, 16)
            sync.wait_ge(stA, 16 * (D4 // 2))
            sync.wait_ge(stB, 16 * (D4 // 2))
            sync.wait_ge(stR, 16 * D4)

    return nc


def _get_nc():
    if "nc" not in _CACHE:
        _CACHE["nc"] = _build_bass()
    return _CACHE["nc"]


def _get_exec():
    """Build and cache the jitted SPMD executable (with output donation) and
    a device-side zero-buffer maker, so repeat kernel() calls only pay
    input upload + execution + output download."""
    if "exec" in _CACHE:
        return _CACHE["exec"]

    import jax
    import jax.numpy as jnp
    from jax.sharding import Mesh, NamedSharding, PartitionSpec
    from jax.experimental.shard_map import shard_map
    import concourse.mybir as mybir
    from concourse import bass2jax

    nc = _get_nc()
    bass2jax.install_neuronx_cc_hook()
    partition_name = nc.partition_id_tensor.name if nc.partition_id_tensor else None

    in_names, out_names, out_avals = [], [], []
    for alloc in nc.m.functions[0].allocations:
        if not isinstance(alloc, mybir.MemoryLocationSet):
            continue
        name = alloc.memorylocations[0].name
        if alloc.kind == "ExternalInput":
            if name != partition_name:
                in_names.append(name)
        elif alloc.kind == "ExternalOutput":
            out_names.append(name)
            out_avals.append(
                jax.core.ShapedArray(tuple(alloc.tensor_shape), mybir.dt.np(alloc.dtype))
            )
    n_params = len(in_names)
    all_names = list(in_names) + out_names
    if partition_name is not None:
        all_names.append(partition_name)

    def _body(*args):
        operands = list(args)
        if partition_name is not None:
            operands.append(bass2jax.partition_id_tensor())
        outs = bass2jax._bass_exec_p.bind(
            *operands,
            out_avals=tuple(out_avals),
            in_names=tuple(all_names),
            out_names=tuple(out_names),
            lowering_input_output_aliases=(),
            sim_require_finite=True,
            sim_require_nnan=True,
            nc=nc,
        )
        return tuple(outs)

    devices = jax.devices()[:N_CORES]
    mesh = Mesh(np.asarray(devices), ("core",))
    spec = PartitionSpec("core")
    n_outs = len(out_names)
    donate = tuple(range(n_params, n_params + n_outs))
    fn = jax.jit(
        shard_map(
            _body,
            mesh=mesh,
            in_specs=(spec,) * (n_params + n_outs),
            out_specs=(spec,) * n_outs,
            check_rep=False,
        ),
        donate_argnums=donate,
        keep_unused=True,
    )

    sharding = NamedSharding(mesh, spec)
    zero_makers = [
        jax.jit(
            lambda aval=aval: jnp.zeros((N_CORES * aval.shape[0], *aval.shape[1:]), aval.dtype),
            out_shardings=sharding,
        )
        for aval in out_avals
    ]
    _CACHE["exec"] = (fn, in_names, zero_makers, sharding)
    return _CACHE["exec"]


def kernel(left_feature, right_feature, max_disparity=192):
    import jax

    assert int(max_disparity) == D4 * 4
    lf = np.ascontiguousarray(np.asarray(left_feature, dtype=np.float32)).reshape(C, H, W)
    rf = np.ascontiguousarray(np.asarray(right_feature, dtype=np.float32)).reshape(C, H, W)

    fn, in_names, zero_makers, sharding = _get_exec()
    # global (concat-over-cores) input arrays; core k's shard is its H-slice
    host_in = {
        "left": lf.transpose(1, 0, 2).reshape(N_CORES, HC, C, W).transpose(0, 2, 1, 3).reshape(N_CORES * C, HC, W),
        "right": rf.transpose(1, 0, 2).reshape(N_CORES, HC, C, W).transpose(0, 2, 1, 3).reshape(N_CORES * C, HC, W),
    }
    args = [jax.device_put(np.ascontiguousarray(host_in[nm]), sharding) for nm in in_names]
    args += [zm() for zm in zero_makers]
    (out_g,) = fn(*args)
    out = np.asarray(out_g)  # (8*64, 48, 16, 240)
    # core k owns H rows [16k, 16k+16): reassemble to (64, 48, 128, 240)
    full = out.reshape(N_CORES, 2 * C, D4, HC, W).transpose(1, 2, 0, 3, 4).reshape(2 * C, D4, H, W)
    return np.ascontiguousarray(full).reshape(1, 2 * C, D4, H, W)
